# revision 11
# baseline (speedup 1.0000x reference)
"""Trainium2 Bass kernel for nn_ASSANetEncoder (point-cloud set-abstraction encoder).

Reference computation per batch b (B=8, N=16384, P=4096, S=32, C_in=64, C=128):
    neigh[c,p,s] = support_features[c, idx[p,s]]
    rel[d,p,s]   = support_xyz[idx[p,s], d] - query_xyz[p, d]
    agg[c,p,s]   = neigh[c,p,s] * rel[d(c),p,s]      (d(c): 21/21/22 repeat-interleave)
    y1 = relu(scale1*(W1@agg) + shift1)              (inference BN folded to scale/shift)
    y2 = scale2*(W2@y1) + shift2
    out[o,p]     = max_s relu(y2)

Sharding: data-parallel over batch, one batch per NeuronCore (8 cores).

Per-core kernel strategy:
  * A per-batch fp16 table with rows [f*g_rep (64ch) | f (64ch)] lives in SBUF;
    rows are fetched with SBUF-source dma_gather(transpose=True), which lands
    gathered rows directly in [channel-on-partition, point-on-free] layout.
  * agg = f*(g_rep - q_rep) is decomposed as f*g_rep - f*q_rep:
      - the f*g_rep product is precomputed per support point (table top half)
      - f*q_rep is one fp16 tensor_tensor multiply on the gathered bottom half
      - the subtraction folds into the first matmul by stacking [W1'; -W1']
        as a K=128 lhsT.
  * BN scales fold into the conv weights; shifts ride the ReLU activations as
    per-partition bias. max over S commutes with the final (bias+relu).
  * Gathers are issued once per s value (4096 indices each) to amortize SWDGE
    descriptor generation; matmuls consume 512-column chunks (one PSUM bank).
"""

import os
import sys

sys.path.insert(0, "/opt/trn_rl_repo")

import numpy as np

B, N, NPOINT, NSAMPLE = 8, 16384, 4096, 32
C_IN, C_MID, C_OUT = 64, 128, 128
EPS = 1e-5
REPEATS = [21, 21, 22]

CHUNK = 512                    # matmul free dim / PSUM bank
NCHUNK = NPOINT // CHUNK       # 8 chunks per s slice
GIDX = int(os.environ.get("K_GIDX", str(NPOINT)))  # indices per dma_gather
TBL_HBM = os.environ.get("K_TBL", "sbuf") == "hbm"  # gather source

_compiled = None


def _build():
    import concourse.tile as tile
    from concourse import bacc, mybir

    f16 = mybir.dt.float16
    f32 = mybir.dt.float32
    i16 = mybir.dt.int16
    Alu = mybir.AluOpType
    Act = mybir.ActivationFunctionType

    nc = bacc.Bacc("TRN2", target_bir_lowering=False, debug=False,
                   enable_asserts=False, num_devices=8)

    if TBL_HBM:
        table_d = nc.dram_tensor("table", [N, 128], f16, kind="ExternalInput")
    else:
        table_d = nc.dram_tensor("table", [128, N], f16, kind="ExternalInput")
        # row-major copy of the table for gather 0: sourcing the first gather
        # from HBM removes its dependency on the 4 MB SBUF table load, which
        # otherwise stalls the GpSimd pipeline ~28 us at startup
        table_hd = nc.dram_tensor("table_h", [N, 128], f16, kind="ExternalInput")
    idx_d = nc.dram_tensor("idx", [128, NPOINT * NSAMPLE // 16], i16,
                           kind="ExternalInput")
    qi_d = nc.dram_tensor("qi", [C_IN, NPOINT], f16, kind="ExternalInput")
    w_d = nc.dram_tensor("wstack", [128, 256], f16, kind="ExternalInput")
    c_d = nc.dram_tensor("consts", [128, 2], f32, kind="ExternalInput")
    out_d = nc.dram_tensor("out", [C_OUT, NPOINT], f32, kind="ExternalOutput")

    with tile.TileContext(nc) as tc:
        with (
            tc.tile_pool(name="const", bufs=1) as cpool,
            tc.tile_pool(name="g", bufs=3) as gpool,
            tc.tile_pool(name="y1r", bufs=4) as rpool,
            tc.tile_pool(name="ps1", bufs=2, space="PSUM") as ps1,
            tc.tile_pool(name="ps2", bufs=4, space="PSUM") as ps2,
        ):
            idx = cpool.tile([128, NPOINT * NSAMPLE // 16], i16, tag="idx")
            # load the first gather's index slice first, with an 8 KB head
            # covering gather 0's first 512-idx part, so the pipeline starts
            # as soon as the DMA queues come up (Tile tracks per-DMA ranges)
            isl = GIDX // 16
            nc.sync.dma_start(idx[:, :32], idx_d.ap()[:, :32])
            nc.sync.dma_start(idx[:, 32:isl], idx_d.ap()[:, 32:isl])
            w = cpool.tile([128, 256], f16, tag="w")
            nc.scalar.dma_start(w[:], w_d.ap()[:])
            consts = cpool.tile([128, 2], f32, tag="consts")
            nc.scalar.dma_start(consts[:], c_d.ap()[:])
            SPG0 = GIDX // NPOINT if GIDX > NPOINT else 1
            qi = cpool.tile([128, SPG0 * NPOINT], f16, tag="qi")
            for r in range(SPG0):
                nc.scalar.dma_start(qi[64:128, r * NPOINT:(r + 1) * NPOINT],
                                    qi_d.ap()[:])
            if not TBL_HBM:
                # gather 0 reads HBM, so the SBUF table only gates gather 1:
                # both queues have ~31 us to move 4 MB behind gather 0
                table = cpool.tile([128, N], f16, tag="table")
                nq = N // 4
                nc.sync.dma_start(table[:, 0:nq], table_d.ap()[:, 0:nq])
                nc.scalar.dma_start(table[:, nq:2 * nq], table_d.ap()[:, nq:2 * nq])
                nc.sync.dma_start(table[:, 2 * nq:3 * nq], table_d.ap()[:, 2 * nq:3 * nq])
                nc.scalar.dma_start(table[:, 3 * nq:], table_d.ap()[:, 3 * nq:])
            nc.sync.dma_start(idx[:, isl:], idx_d.ap()[:, isl:])
            acc = cpool.tile([128, NPOINT], f32, tag="acc")
            outt = cpool.tile([128, NPOINT], f32, tag="outt")

            nreg = nc.gpsimd.to_reg(GIDX)

            NG = NPOINT * NSAMPLE // GIDX
            SPG = GIDX // NPOINT if GIDX > NPOINT else 1  # s-slices per gather
            nregs = {GIDX: nreg,
                     GIDX // 2: nc.gpsimd.to_reg(GIDX // 2),
                     GIDX // 4: nc.gpsimd.to_reg(GIDX // 4),
                     GIDX // 8: nc.gpsimd.to_reg(GIDX // 8)}
            for g in range(NG):
                G = gpool.tile([128, GIDX], f16, tag="G")
                hbm_src = TBL_HBM or g == 0
                src = table_d.ap()[:] if TBL_HBM else (
                    table_hd.ap()[:] if g == 0 else table[:])
                kw = {} if hbm_src else dict(sbuf_tokens_per_rank=128,
                                             sbuf_free_dim_per_rank=256)
                # gather 0 ramps up in ascending parts (its first part only
                # gates on the 8 KB idx head); the last gather is split into
                # quarters then eighths so the tail chunk pipeline drains
                # incrementally instead of waiting for the full final gather
                if g == 0:
                    parts = [(0, GIDX // 8), (GIDX // 8, GIDX)]
                elif g < NG - 1:
                    parts = [(0, GIDX)]
                else:
                    parts = [(0, GIDX // 4), (GIDX // 4, GIDX // 2),
                             (GIDX // 2, 3 * GIDX // 4),
                             (3 * GIDX // 4, 7 * GIDX // 8),
                             (7 * GIDX // 8, GIDX)]
                for (lo, hi) in parts:
                    nc.gpsimd.dma_gather(
                        G[:, lo:hi].rearrange("p (a n) -> p a n", a=1),
                        src,
                        idx[:, (g * GIDX + lo) // 16:(g * GIDX + hi) // 16],
                        hi - lo,
                        nregs[hi - lo],
                        128,
                        transpose=True,
                        # single_packet=True packs all descriptors into one
                        # DMA packet; beyond ~64 descriptors that wedges the
                        # device.
                        single_packet=False,
                        **kw,
                    )
                    # bottom half: f * q_rep (in place) per gathered range
                    nc.vector.tensor_tensor(G[64:128, lo:hi], G[64:128, lo:hi],
                                            qi[64:128, lo:hi], Alu.mult)
                for cc in range(GIDX // CHUNK):
                    s = g * GIDX // NPOINT + cc * CHUNK // NPOINT
                    c = cc % NCHUNK
                    cs = slice(cc * CHUNK, (cc + 1) * CHUNK)
                    acs = slice((c * CHUNK) % NPOINT, (c * CHUNK) % NPOINT + CHUNK)
                    y1 = ps1.tile([128, CHUNK], f32, tag="y1")
                    nc.tensor.matmul(y1[:], w[:, 0:128], G[:, cs],
                                     start=True, stop=True)
                    y1r = rpool.tile([128, CHUNK], f16, tag="y1r")
                    nc.scalar.activation(y1r[:], y1[:], Act.Relu,
                                         bias=consts[:, 0:1], scale=1.0)
                    y2 = ps2.tile([128, CHUNK], f32, tag="y2")
                    nc.tensor.matmul(y2[:], w[:, 128:256], y1r[:],
                                     start=True, stop=True)
                    if s == 0:
                        nc.scalar.activation(acc[:, acs], y2[:], Act.Copy)
                    else:
                        nc.vector.tensor_tensor(acc[:, acs], y2[:], acc[:, acs],
                                                Alu.max)
                    if s == NSAMPLE - 1:
                        nc.scalar.activation(outt[:, acs], acc[:, acs], Act.Relu,
                                             bias=consts[:, 1:2], scale=1.0)
                        if cc == GIDX // CHUNK - 1:
                            # final chunk: split the 256 KB writeback across
                            # two queues to shorten the drain
                            h = slice(acs.start, acs.start + CHUNK // 2)
                            h2 = slice(acs.start + CHUNK // 2, acs.stop)
                            nc.sync.dma_start(out_d.ap()[:, h], outt[:, h])
                            nc.scalar.dma_start(out_d.ap()[:, h2], outt[:, h2])
                        else:
                            nc.sync.dma_start(out_d.ap()[:, acs], outt[:, acs])

    nc.compile()
    return nc


def _get_compiled():
    global _compiled
    if _compiled is None:
        _compiled = _build()
    return _compiled


def _prep_core_inputs(b, query_xyz, support_xyz, support_features, neighbor_idx,
                      wstack, consts):
    f = np.asarray(support_features[b], np.float32)            # [64, N]
    grep = np.repeat(np.asarray(support_xyz[b], np.float32).T,
                     REPEATS, axis=0)                          # [64, N]
    rows = np.concatenate([(f * grep).T, f.T], axis=1).astype(np.float16)
    table_h = np.ascontiguousarray(rows)                 # [N, 128] DRAM rows
    if TBL_HBM:
        table = table_h
    else:
        # SBUF layout: partition = row % 128, rank (free 256B slot) = row // 128
        table = np.ascontiguousarray(
            rows.reshape(N // 128, 128, 128).transpose(1, 0, 2).reshape(128, N))

    stream = np.asarray(neighbor_idx[b], np.int64).T.reshape(-1)  # [S*P], p fastest
    wrapped = stream.astype(np.int16).reshape(-1, 16).T           # [16, S*P/16]
    idx = np.ascontiguousarray(np.tile(wrapped, (8, 1)))          # [128, S*P/16]

    qi = np.ascontiguousarray(
        np.repeat(np.asarray(query_xyz[b], np.float32).T, REPEATS, axis=0)
    ).astype(np.float16)                                          # [64, P]

    m = {"table": table, "idx": idx, "qi": qi,
         "wstack": wstack, "consts": consts}
    if not TBL_HBM:
        m["table_h"] = table_h
    return m


def _ensure_trace_shim():
    """If BASS_TRACE is set but this image lacks antenv.axon_hooks, install a
    working shim (or a no-op) so run_bass_kernel_spmd never crashes."""
    try:
        import antenv.axon_hooks  # noqa: F401
        return
    except ImportError:
        pass
    import types
    import antenv
    hook = None
    try:
        from trn_agent_boot import trn_boot
        hook = trn_boot._ntff_profile_via_ctypes("/opt/axon/libaxon_pjrt.so")
    except Exception:
        hook = None
    shim = types.ModuleType("antenv.axon_hooks")
    shim.get_axon_ntff_profile_hook = lambda: hook
    shim.set_axon_ntff_profile_hook = lambda h: None
    sys.modules["antenv.axon_hooks"] = shim
    antenv.axon_hooks = shim


def kernel(query_xyz, support_xyz, support_features, neighbor_idx,
           W1, g1, b1, m1, v1, W2, g2, b2, m2, v2):
    from concourse.bass_utils import run_bass_kernel_spmd

    _ensure_trace_shim()

    nc = _get_compiled()

    scale1 = np.asarray(g1, np.float32) / np.sqrt(np.asarray(v1, np.float32) + EPS)
    shift1 = np.asarray(b1, np.float32) - np.asarray(m1, np.float32) * scale1
    scale2 = np.asarray(g2, np.float32) / np.sqrt(np.asarray(v2, np.float32) + EPS)
    shift2 = np.asarray(b2, np.float32) - np.asarray(m2, np.float32) * scale2

    W1p = (scale1[:, None] * np.asarray(W1, np.float32)).T     # [64, 128] lhsT
    W2p = (scale2[:, None] * np.asarray(W2, np.float32)).T     # [128, 128] lhsT
    lhsT1 = np.concatenate([W1p, -W1p], axis=0)                # [128, 128]
    wstack = np.ascontiguousarray(
        np.concatenate([lhsT1, W2p], axis=1)).astype(np.float16)
    consts = np.ascontiguousarray(np.stack([shift1, shift2], axis=1),
                                  dtype=np.float32)

    in_maps = [
        _prep_core_inputs(b, query_xyz, support_xyz, support_features,
                          neighbor_idx, wstack, consts)
        for b in range(B)
    ]

    res = run_bass_kernel_spmd(nc, in_maps, core_ids=list(range(B)))
    out = np.stack([res.results[b]["out"] for b in range(B)], axis=0)
    kernel.last_results = res
    return out.astype(np.float32)



# revision 13
# speedup vs baseline: 1.1315x; 1.1315x over previous
"""Trainium2 Bass kernel for nn_ASSANetEncoder (point-cloud set-abstraction encoder).

Reference computation per batch b (B=8, N=16384, P=4096, S=32, C_in=64, C=128):
    neigh[c,p,s] = support_features[c, idx[p,s]]
    rel[d,p,s]   = support_xyz[idx[p,s], d] - query_xyz[p, d]
    agg[c,p,s]   = neigh[c,p,s] * rel[d(c),p,s]      (d(c): 21/21/22 repeat-interleave)
    y1 = relu(scale1*(W1@agg) + shift1)              (inference BN folded to scale/shift)
    y2 = scale2*(W2@y1) + shift2
    out[o,p]     = max_s relu(y2)

Sharding: data-parallel over batch, one batch per NeuronCore (8 cores).

Per-core kernel strategy:
  * A per-batch fp16 table with rows [f*g_rep (64ch) | f (64ch)] lives in SBUF;
    rows are fetched with SBUF-source dma_gather(transpose=True), which lands
    gathered rows directly in [channel-on-partition, point-on-free] layout.
  * agg = f*(g_rep - q_rep) is decomposed as f*g_rep - f*q_rep:
      - the f*g_rep product is precomputed per support point (table top half)
      - f*q_rep is one fp16 tensor_tensor multiply on the gathered bottom half
      - the subtraction folds into the first matmul by stacking [W1'; -W1']
        as a K=128 lhsT.
  * BN scales fold into the conv weights; shifts ride the ReLU activations as
    per-partition bias. max over S commutes with the final (bias+relu).
  * Gathers are issued once per s value (4096 indices each) to amortize SWDGE
    descriptor generation; matmuls consume 512-column chunks (one PSUM bank).
    GpSimd descriptor generation (~7.7 ns/index) is the measured hardware
    bottleneck; everything else overlaps underneath it. Larger gathers (8192)
    overflow the SWDGE ring and stall; ap_gather / indirect_copy are slower.
  * Startup: gather 0 sources a row-major HBM copy of the table, so it only
    gates on an 8 KB index-head DMA instead of the 4 MB SBUF table load.
    Tail: the last gather is split into quarters/eighths and the final
    writeback across two queues so the pipeline drains incrementally.
"""

import os
import sys

sys.path.insert(0, "/opt/trn_rl_repo")

import numpy as np

B, N, NPOINT, NSAMPLE = 8, 16384, 4096, 32
C_IN, C_MID, C_OUT = 64, 128, 128
EPS = 1e-5
REPEATS = [21, 21, 22]

CHUNK = 512                    # matmul free dim / PSUM bank
NCHUNK = NPOINT // CHUNK       # 8 chunks per s slice
GIDX = int(os.environ.get("K_GIDX", str(NPOINT)))  # indices per dma_gather
TBL_HBM = os.environ.get("K_TBL", "sbuf") == "hbm"  # gather source

_compiled = None


def _build():
    import concourse.tile as tile
    from concourse import bacc, mybir

    f16 = mybir.dt.float16
    f32 = mybir.dt.float32
    i16 = mybir.dt.int16
    Alu = mybir.AluOpType
    Act = mybir.ActivationFunctionType

    nc = bacc.Bacc("TRN2", target_bir_lowering=False, debug=False,
                   enable_asserts=False, num_devices=8)

    if TBL_HBM:
        table_d = nc.dram_tensor("table", [N, 128], f16, kind="ExternalInput")
    else:
        table_d = nc.dram_tensor("table", [128, N], f16, kind="ExternalInput")
        # row-major copy of the table for gather 0: sourcing the first gather
        # from HBM removes its dependency on the 4 MB SBUF table load, which
        # otherwise stalls the GpSimd pipeline ~28 us at startup
        table_hd = nc.dram_tensor("table_h", [N, 128], f16, kind="ExternalInput")
    idx_d = nc.dram_tensor("idx", [128, NPOINT * NSAMPLE // 16], i16,
                           kind="ExternalInput")
    qi_d = nc.dram_tensor("qi", [C_IN, NPOINT], f16, kind="ExternalInput")
    w_d = nc.dram_tensor("wstack", [128, 256], f16, kind="ExternalInput")
    c_d = nc.dram_tensor("consts", [128, 2], f32, kind="ExternalInput")
    out_d = nc.dram_tensor("out", [C_OUT, NPOINT], f32, kind="ExternalOutput")

    with tile.TileContext(nc) as tc:
        with (
            tc.tile_pool(name="const", bufs=1) as cpool,
            tc.tile_pool(name="g", bufs=3) as gpool,
            tc.tile_pool(name="y1r", bufs=4) as rpool,
            tc.tile_pool(name="ps1", bufs=2, space="PSUM") as ps1,
            tc.tile_pool(name="ps2", bufs=4, space="PSUM") as ps2,
        ):
            idx = cpool.tile([128, NPOINT * NSAMPLE // 16], i16, tag="idx")
            # load the first gather's index slice first, with an 8 KB head
            # covering gather 0's first 512-idx part, so the pipeline starts
            # as soon as the DMA queues come up (Tile tracks per-DMA ranges)
            isl = GIDX // 16
            nc.sync.dma_start(idx[:, :32], idx_d.ap()[:, :32])
            nc.sync.dma_start(idx[:, 32:isl], idx_d.ap()[:, 32:isl])
            w = cpool.tile([128, 256], f16, tag="w")
            nc.scalar.dma_start(w[:], w_d.ap()[:])
            consts = cpool.tile([128, 2], f32, tag="consts")
            nc.scalar.dma_start(consts[:], c_d.ap()[:])
            SPG0 = GIDX // NPOINT if GIDX > NPOINT else 1
            qi = cpool.tile([128, SPG0 * NPOINT], f16, tag="qi")
            for r in range(SPG0):
                nc.scalar.dma_start(qi[64:128, r * NPOINT:(r + 1) * NPOINT],
                                    qi_d.ap()[:])
            if not TBL_HBM:
                # gather 0 reads HBM, so the SBUF table only gates gather 1:
                # both queues have ~31 us to move 4 MB behind gather 0
                table = cpool.tile([128, N], f16, tag="table")
                nq = N // 4
                nc.sync.dma_start(table[:, 0:nq], table_d.ap()[:, 0:nq])
                nc.scalar.dma_start(table[:, nq:2 * nq], table_d.ap()[:, nq:2 * nq])
                nc.sync.dma_start(table[:, 2 * nq:3 * nq], table_d.ap()[:, 2 * nq:3 * nq])
                nc.scalar.dma_start(table[:, 3 * nq:], table_d.ap()[:, 3 * nq:])
            nc.sync.dma_start(idx[:, isl:], idx_d.ap()[:, isl:])
            acc = cpool.tile([128, NPOINT], f32, tag="acc")
            outt = cpool.tile([128, NPOINT], f32, tag="outt")

            nreg = nc.gpsimd.to_reg(GIDX)

            NG = NPOINT * NSAMPLE // GIDX
            SPG = GIDX // NPOINT if GIDX > NPOINT else 1  # s-slices per gather
            nregs = {GIDX: nreg,
                     7 * GIDX // 8: nc.gpsimd.to_reg(7 * GIDX // 8),
                     GIDX // 4: nc.gpsimd.to_reg(GIDX // 4),
                     GIDX // 8: nc.gpsimd.to_reg(GIDX // 8)}
            for g in range(NG):
                G = gpool.tile([128, GIDX], f16, tag="G")
                hbm_src = TBL_HBM or g == 0
                src = table_d.ap()[:] if TBL_HBM else (
                    table_hd.ap()[:] if g == 0 else table[:])
                kw = {} if hbm_src else dict(sbuf_tokens_per_rank=128,
                                             sbuf_free_dim_per_rank=256)
                # gather 0 ramps up in ascending parts (its first part only
                # gates on the 8 KB idx head); the last gather is split into
                # quarters then eighths so the tail chunk pipeline drains
                # incrementally instead of waiting for the full final gather
                if g == 0:
                    parts = [(0, GIDX // 8), (GIDX // 8, GIDX)]
                elif g < NG - 1:
                    parts = [(0, GIDX)]
                else:
                    parts = [(0, GIDX // 4), (GIDX // 4, GIDX // 2),
                             (GIDX // 2, 3 * GIDX // 4),
                             (3 * GIDX // 4, 7 * GIDX // 8),
                             (7 * GIDX // 8, GIDX)]
                for (lo, hi) in parts:
                    nc.gpsimd.dma_gather(
                        G[:, lo:hi].rearrange("p (a n) -> p a n", a=1),
                        src,
                        idx[:, (g * GIDX + lo) // 16:(g * GIDX + hi) // 16],
                        hi - lo,
                        nregs[hi - lo],
                        128,
                        transpose=True,
                        # single_packet=True packs all descriptors into one
                        # DMA packet; beyond ~64 descriptors that wedges the
                        # device.
                        single_packet=False,
                        **kw,
                    )
                    # bottom half: f * q_rep (in place) per gathered range
                    nc.vector.tensor_tensor(G[64:128, lo:hi], G[64:128, lo:hi],
                                            qi[64:128, lo:hi], Alu.mult)
                for cc in range(GIDX // CHUNK):
                    s = g * GIDX // NPOINT + cc * CHUNK // NPOINT
                    c = cc % NCHUNK
                    cs = slice(cc * CHUNK, (cc + 1) * CHUNK)
                    acs = slice((c * CHUNK) % NPOINT, (c * CHUNK) % NPOINT + CHUNK)
                    y1 = ps1.tile([128, CHUNK], f32, tag="y1")
                    nc.tensor.matmul(y1[:], w[:, 0:128], G[:, cs],
                                     start=True, stop=True)
                    y1r = rpool.tile([128, CHUNK], f16, tag="y1r")
                    nc.scalar.activation(y1r[:], y1[:], Act.Relu,
                                         bias=consts[:, 0:1], scale=1.0)
                    y2 = ps2.tile([128, CHUNK], f32, tag="y2")
                    nc.tensor.matmul(y2[:], w[:, 128:256], y1r[:],
                                     start=True, stop=True)
                    if s == 0:
                        nc.scalar.activation(acc[:, acs], y2[:], Act.Copy)
                    else:
                        nc.vector.tensor_tensor(acc[:, acs], y2[:], acc[:, acs],
                                                Alu.max)
                    if s == NSAMPLE - 1:
                        nc.scalar.activation(outt[:, acs], acc[:, acs], Act.Relu,
                                             bias=consts[:, 1:2], scale=1.0)
                        if cc == GIDX // CHUNK - 1:
                            # final chunk: split the 256 KB writeback across
                            # two queues to shorten the drain
                            h = slice(acs.start, acs.start + CHUNK // 2)
                            h2 = slice(acs.start + CHUNK // 2, acs.stop)
                            nc.sync.dma_start(out_d.ap()[:, h], outt[:, h])
                            nc.scalar.dma_start(out_d.ap()[:, h2], outt[:, h2])
                        else:
                            nc.sync.dma_start(out_d.ap()[:, acs], outt[:, acs])

    nc.compile()
    return nc


def _get_compiled():
    global _compiled
    if _compiled is None:
        _compiled = _build()
    return _compiled


def _prep_core_inputs(b, query_xyz, support_xyz, support_features, neighbor_idx,
                      wstack, consts):
    f = np.asarray(support_features[b], np.float32)            # [64, N]
    grep = np.repeat(np.asarray(support_xyz[b], np.float32).T,
                     REPEATS, axis=0)                          # [64, N]
    rows = np.concatenate([(f * grep).T, f.T], axis=1).astype(np.float16)
    table_h = np.ascontiguousarray(rows)                 # [N, 128] DRAM rows
    if TBL_HBM:
        table = table_h
    else:
        # SBUF layout: partition = row % 128, rank (free 256B slot) = row // 128
        table = np.ascontiguousarray(
            rows.reshape(N // 128, 128, 128).transpose(1, 0, 2).reshape(128, N))

    stream = np.asarray(neighbor_idx[b], np.int64).T.reshape(-1)  # [S*P], p fastest
    wrapped = stream.astype(np.int16).reshape(-1, 16).T           # [16, S*P/16]
    idx = np.ascontiguousarray(np.tile(wrapped, (8, 1)))          # [128, S*P/16]

    qi = np.ascontiguousarray(
        np.repeat(np.asarray(query_xyz[b], np.float32).T, REPEATS, axis=0)
    ).astype(np.float16)                                          # [64, P]

    m = {"table": table, "idx": idx, "qi": qi,
         "wstack": wstack, "consts": consts}
    if not TBL_HBM:
        m["table_h"] = table_h
    return m


def _ensure_trace_shim():
    """If BASS_TRACE is set but this image lacks antenv.axon_hooks, install a
    working shim (or a no-op) so run_bass_kernel_spmd never crashes."""
    try:
        import antenv.axon_hooks  # noqa: F401
        return
    except ImportError:
        pass
    import types
    import antenv
    hook = None
    try:
        from trn_agent_boot import trn_boot
        hook = trn_boot._ntff_profile_via_ctypes("/opt/axon/libaxon_pjrt.so")
    except Exception:
        hook = None
    shim = types.ModuleType("antenv.axon_hooks")
    shim.get_axon_ntff_profile_hook = lambda: hook
    shim.set_axon_ntff_profile_hook = lambda h: None
    sys.modules["antenv.axon_hooks"] = shim
    antenv.axon_hooks = shim


def kernel(query_xyz, support_xyz, support_features, neighbor_idx,
           W1, g1, b1, m1, v1, W2, g2, b2, m2, v2):
    from concourse.bass_utils import run_bass_kernel_spmd

    _ensure_trace_shim()

    nc = _get_compiled()

    scale1 = np.asarray(g1, np.float32) / np.sqrt(np.asarray(v1, np.float32) + EPS)
    shift1 = np.asarray(b1, np.float32) - np.asarray(m1, np.float32) * scale1
    scale2 = np.asarray(g2, np.float32) / np.sqrt(np.asarray(v2, np.float32) + EPS)
    shift2 = np.asarray(b2, np.float32) - np.asarray(m2, np.float32) * scale2

    W1p = (scale1[:, None] * np.asarray(W1, np.float32)).T     # [64, 128] lhsT
    W2p = (scale2[:, None] * np.asarray(W2, np.float32)).T     # [128, 128] lhsT
    lhsT1 = np.concatenate([W1p, -W1p], axis=0)                # [128, 128]
    wstack = np.ascontiguousarray(
        np.concatenate([lhsT1, W2p], axis=1)).astype(np.float16)
    consts = np.ascontiguousarray(np.stack([shift1, shift2], axis=1),
                                  dtype=np.float32)

    in_maps = [
        _prep_core_inputs(b, query_xyz, support_xyz, support_features,
                          neighbor_idx, wstack, consts)
        for b in range(B)
    ]

    res = run_bass_kernel_spmd(nc, in_maps, core_ids=list(range(B)))
    out = np.stack([res.results[b]["out"] for b in range(B)], axis=0)
    kernel.last_results = res
    return out.astype(np.float32)

